# revision 1
# baseline (speedup 1.0000x reference)
"""Trainium2 Bass kernel for nn_Net_84275848282731 (3-layer 4-edge-type GAT).

Design (8 NeuronCores, SPMD):
  - Nodes are range-sharded over cores (6250/core); each core owns the edges
    whose dst falls in its shard (edge/dst parallel).
  - Per layer: device computes H_ext = x @ W_ext per node shard (W_ext packs
    [H | es | ed | pad] columns for the 4 edge classes + the skip projection;
    BatchNorm is folded into W_ext on-device from AllReduced stats), then
    AllGathers the per-(node,class) feature table, then processes its edge
    shard: for each 128-edge tile, indirect-DMA gather of table rows by
    (src,class), attention weights w = exp(leaky_relu(es_src + ed_dst)), and
    a one-hot matmul accumulates softmax numerator+denominator per
    (dst-chunk, class) in PSUM.  Per chunk: out = skip + sum_j msg_j/den_j,
    relu, store; BN stats for the next layer accumulate on the fly.
  - Host does the edge sort/padding and packs per-tile metadata; the bass
    program structure (segment tile counts) is shared across cores (padded to
    the max over cores), per-core behavior differs only via input data.
"""
import numpy as np

import concourse.bass as bass
import concourse.bacc as bacc
import concourse.tile as tile
import concourse.mybir as mybir
from concourse.bass_utils import run_bass_kernel_spmd

F32 = mybir.dt.float32
I32 = mybir.dt.int32
ET = mybir.EngineType
AF = mybir.ActivationFunctionType
OP = mybir.AluOpType

NCORES = 8
N, E, F_IN = 50000, 800000, 64
HID, HEADS = 64, 4
NEG = 0.2
EPS = 1e-5
P = 128

RPC = N // NCORES              # real nodes per core: 6250
NCHUNK = (RPC + P - 1) // P    # 49
RPAD = NCHUNK * P              # 6272 padded nodes per core
TROWS = RPAD * 4 + 4           # table rows per core shard: 25092 (incl 4 dummy)
DUMMY = RPAD * 4               # local dummy row index

# per-class column block in the table row: [H(256)|es(4)|ed(4)|pad(8)] = 272
CB = 272
ROW01 = CB                     # 272 table row f32 for layers 0/1
ROW2 = 4                       # layer2 row: [H(2)|es|ed]
COLS01 = 4 * CB + 256          # dense output cols for layers 0/1 (incl skip)
COLS2 = 4 * ROW2 + 256         # layer2 dense cols

_BUILD_CACHE = {}


# --------------------------------------------------------------------------
# host-side preprocessing
# --------------------------------------------------------------------------

def _prep_edges(edge_index, edge_class):
    """Per-core edge shards sorted by (chunk, class), padded so that the
    segment structure (tiles per (chunk,class)) is identical across cores.

    Returns (segs, per_core) where segs = [(chunk, class, ntiles), ...] and
    per_core[c] = dict(rowidx[T,128] int32, dstrow[T,128] float32).
    """
    src, dst = np.asarray(edge_index[0]), np.asarray(edge_index[1])
    k = np.asarray(edge_class)
    percore = []
    counts = np.zeros((NCORES, NCHUNK, 4), np.int64)
    for c in range(NCORES):
        lo = c * RPC
        m = (dst >= lo) & (dst < lo + RPC)
        s, d, kk = src[m], dst[m] - lo, k[m]
        ch = d // P
        order = np.lexsort((d, kk, ch))
        s, d, kk, ch = s[order], d[order], kk[order], ch[order]
        percore.append((s, d, kk, ch))
        cc = np.zeros((NCHUNK, 4), np.int64)
        np.add.at(cc, (ch, kk), 1)
        counts[c] = cc
    ntiles = (counts.max(axis=0) + P - 1) // P          # [NCHUNK, 4]
    segs = [(ch, j, int(ntiles[ch, j])) for ch in range(NCHUNK) for j in range(4)
            if ntiles[ch, j] > 0]
    T = int(sum(t for _, _, t in segs))

    out = []
    for c in range(NCORES):
        s, d, kk, ch = percore[c]
        rowidx = np.full((T, P), DUMMY, np.int64)   # local dummy of rank 0 region
        dstrow = np.zeros((T, P), np.float32)
        tptr = 0
        pos = 0
        # edges are sorted by (ch, kk, d); walk segments in the same order
        for (chu, j, nt) in segs:
            cnt = int(counts[c, chu, j])
            seg_s = s[pos:pos + cnt]
            seg_d = d[pos:pos + cnt]
            pos += cnt
            srank = seg_s // RPC
            sloc = seg_s - srank * RPC
            gl_rows = srank * TROWS + sloc * 4 + j
            for t in range(nt):
                a, b = t * P, min((t + 1) * P, cnt)
                if a < b:
                    rowidx[tptr + t, :b - a] = gl_rows[a:b]
                    dstrow[tptr + t, :b - a] = (seg_d[a:b] - chu * P).astype(np.float32)
            tptr += nt
        assert pos == len(s)
        out.append(dict(rowidx=rowidx.astype(np.int32), dstrow=dstrow))
    return segs, out


def _host_weights(inputs):
    """Pack W_ext per layer: [din, COLS] with per-class blocks
    [W_j | W_j@As_j | W_j@Ad_j | pad] then skip block, plus const bias rows."""
    packs = []
    for i, (din, heads, dout) in enumerate([(F_IN, 4, 64), (256, 4, 64), (256, 1, 2)]):
        W = np.asarray(inputs[f"W{i}"], np.float32)      # [4, din, heads*dout]
        As = np.asarray(inputs[f"As{i}"], np.float32)    # [4, heads, dout]
        Ad = np.asarray(inputs[f"Ad{i}"], np.float32)
        cb = np.asarray(inputs[f"cb{i}"], np.float32)
        skW = np.asarray(inputs[f"skW{i}"], np.float32)
        skb = np.asarray(inputs[f"skb{i}"], np.float32)
        hd = heads * dout
        cbk = CB if i < 2 else 4
        cols = 4 * cbk + 256 if i < 2 else 4 * cbk + 256
        Wx = np.zeros((din, cols), np.float32)
        for j in range(4):
            b = j * cbk
            Wj = W[j]
            wAs = np.einsum("dhk,hk->dh", Wj.reshape(din, heads, dout), As[j])
            wAd = np.einsum("dhk,hk->dh", Wj.reshape(din, heads, dout), Ad[j])
            Wx[:, b:b + hd] = Wj
            Wx[:, b + hd:b + hd + heads] = wAs
            Wx[:, b + hd + heads:b + hd + 2 * heads] = wAd
        Wx[:, 4 * cbk:4 * cbk + 256] = np.pad(skW, ((0, 0), (0, 256 - skW.shape[1])))
        cr = np.zeros((1, cols), np.float32)
        cr[0, 4 * cbk:4 * cbk + skW.shape[1]] = skb + cb.sum(0)
        packs.append((Wx, cr))
    return packs


# --------------------------------------------------------------------------
# device kernel builder
# --------------------------------------------------------------------------

def build_kernel(segs, reps=1):
    nc = bacc.Bacc("TRN2", target_bir_lowering=False, debug=False, num_devices=NCORES)
    T = sum(t for _, _, t in segs)

    # ---------------- inputs ----------------
    xn0_r = nc.dram_tensor("xn0", [RPAD, F_IN], F32, kind="ExternalInput")
    meta_r = nc.dram_tensor("meta", [T, P, 2], I32, kind="ExternalInput")
    dstrow_r = nc.dram_tensor("dstrow", [T, P], F32, kind="ExternalInput")
    w_r = [nc.dram_tensor(f"wext{i}", [(F_IN if i == 0 else 256), (COLS01 if i < 2 else COLS2)],
                          F32, kind="ExternalInput") for i in range(3)]
    cr_r = [nc.dram_tensor(f"crow{i}", [1, (COLS01 if i < 2 else COLS2)], F32,
                           kind="ExternalInput") for i in range(3)]
    bn_r = [nc.dram_tensor(f"bn{i}", [1, 512], F32, kind="ExternalInput")
            for i in (1, 2)]  # [gamma|beta] for layers 1,2
    out_r = nc.dram_tensor("out", [RPAD, 2], F32, kind="ExternalOutput")

    # ---------------- scratch DRAM ----------------
    tbl_shard = nc.dram_tensor("tbl_shard", [TROWS, ROW01], F32, kind="Internal")
    tbl_full = nc.dram_tensor("tbl_full", [NCORES * TROWS, ROW01], F32,
                              kind="Internal", addr_space="Shared")
    tbl2_shard = nc.dram_tensor("tbl2_shard", [TROWS, ROW2], F32, kind="Internal")
    tbl2_full = nc.dram_tensor("tbl2_full", [NCORES * TROWS, ROW2], F32,
                               kind="Internal", addr_space="Shared")
    x_bufs = [nc.dram_tensor(f"xbuf{i}", [RPAD, 256], F32, kind="Internal")
              for i in range(2)]
    x0_buf = nc.dram_tensor("x0buf", [RPAD, 256], F32, kind="Internal")
    st_in = nc.dram_tensor("stin", [1, 512], F32, kind="Internal")
    st_out = nc.dram_tensor("stout", [1, 512], F32, kind="Internal", addr_space="Shared")
    rg = [list(range(NCORES))]

    with tile.TileContext(nc) as tc:
        with (
            tc.tile_pool(name="const", bufs=1) as constp,
            tc.tile_pool(name="wpool", bufs=1) as wpool,
            tc.tile_pool(name="dense", bufs=3) as dense,
            tc.tile_pool(name="dpsum", bufs=1, space="PSUM") as dpsum,
            tc.tile_pool(name="edge", bufs=6) as edge,
            tc.tile_pool(name="apsum", bufs=2, space="PSUM") as apsum,
            tc.tile_pool(name="epsum", bufs=1, space="PSUM") as epsum,
            tc.tile_pool(name="stat", bufs=1) as statp,
            tc.tile_pool(name="chnk", bufs=3) as chnk,
        ):
            iota_row = constp.tile([P, P], F32)      # [p, f] = f
            iota_col = constp.tile([P, 1], F32)      # [p, 0] = p
            ones_row = constp.tile([1, P], F32)
            ones_col = constp.tile([P, 1], F32)
            ii = constp.tile([P, P], I32)
            nc.gpsimd.iota(ii[:], pattern=[[1, P]], base=0, channel_multiplier=0)
            nc.vector.tensor_copy(iota_row[:], ii[:])
            ic = constp.tile([P, 1], I32)
            nc.gpsimd.iota(ic[:], pattern=[[0, 1]], base=0, channel_multiplier=1)
            nc.vector.tensor_copy(iota_col[:], ic[:])
            nc.vector.memset(ones_row[:], 1.0)
            nc.vector.memset(ones_col[:], 1.0)
            eps_c = constp.tile([1, 1], F32)
            nc.vector.memset(eps_c[:], EPS)
            identity = constp.tile([P, P], F32)
            nc.vector.memset(identity[:], 0.0)
            nc.vector.tensor_scalar(out=identity[:], in0=iota_row[:],
                                    scalar1=iota_col[:, :1], scalar2=None,
                                    op0=OP.is_equal)

            def layer(i, reuse_stats_from=None):
                din = F_IN if i == 0 else 256
                cols = COLS01 if i < 2 else COLS2
                cbk = CB if i < 2 else 4
                heads = 4 if i < 2 else 1
                hd = 256 if i < 2 else 2
                rowlen = ROW01 if i < 2 else ROW2
                tshard = tbl_shard if i < 2 else tbl2_shard
                tfull = tbl_full if i < 2 else tbl2_full
                x_in = xn0_r if i == 0 else x_bufs[(i + 1) % 2]
                x_out = x_bufs[i % 2]
                kt = din // P if din % P == 0 else din // P + 1  # 1 or 2
                nb = (cols + 447) // 448                          # n-tiles of 448

                # ---- stage A: fold BN into weights ----
                wt = wpool.tile([P, 2 * 1408], F32, tag="wt")  # scaled W, k-tile p0 at cols [p0*1408,...)
                bias_row = wpool.tile([1, 1408], F32, tag="bias")
                if i == 0:
                    for p0 in range(kt):
                        pr = min(P, din - p0 * P)
                        nc.sync.dma_start(wt[:pr, p0 * 1408:p0 * 1408 + cols],
                                          w_r[i][p0 * P:p0 * P + pr, :])
                    nc.sync.dma_start(bias_row[:, :cols], cr_r[i][:, :])
                else:
                    # stats from previous layer are in st_out ([1,512]: sum|sumsq)
                    gam = dense.tile([1, 512], F32, tag="gb")
                    nc.sync.dma_start(gam[:], bn_r[i - 1][:, :])
                    srow = dense.tile([1, 512], F32, tag="srow")
                    nc.sync.dma_start(srow[:], st_out[:, :])
                    mean = dense.tile([1, 256], F32, tag="mean")
                    nc.vector.tensor_scalar_mul(mean[:], srow[:, 0:256], 1.0 / N)
                    var = dense.tile([1, 256], F32, tag="var")
                    nc.vector.tensor_scalar_mul(var[:], srow[:, 256:512], 1.0 / N)
                    m2 = dense.tile([1, 256], F32, tag="m2")
                    nc.vector.tensor_tensor(out=m2[:], in0=mean[:], in1=mean[:], op=OP.mult)
                    nc.vector.tensor_tensor(out=var[:], in0=var[:], in1=m2[:], op=OP.subtract)
                    nc.scalar.activation(var[:], var[:], AF.Sqrt, bias=eps_c[:1, :1])
                    nc.vector.reciprocal(var[:], var[:])            # rstd [1,256]
                    srow_s = dense.tile([1, 256], F32, tag="srow_s")  # s = gamma*rstd
                    nc.vector.tensor_tensor(out=srow_s[:], in0=gam[:, 0:256], in1=var[:], op=OP.mult)
                    trow = dense.tile([1, 256], F32, tag="trow")      # t = beta - mean*s
                    nc.vector.tensor_tensor(out=trow[:], in0=mean[:], in1=srow_s[:], op=OP.mult)
                    nc.vector.tensor_tensor(out=trow[:], in0=gam[:, 256:512], in1=trow[:], op=OP.subtract)
                    # transpose s,t to columns via k=1 matmuls
                    scol = dense.tile([P, 4], F32, tag="scol")   # p0 block: [s|t] at cols 2*p0
                    for p0 in range(2):
                        st_ps = dpsum.tile([P, 4], F32, tag="tps")
                        nc.tensor.matmul(out=st_ps[:, 0:1], lhsT=srow_s[:, p0 * P:(p0 + 1) * P],
                                         rhs=ones_row[:1, 0:1], start=True, stop=True)
                        nc.tensor.matmul(out=st_ps[:, 1:2], lhsT=trow[:, p0 * P:(p0 + 1) * P],
                                         rhs=ones_row[:1, 0:1], start=True, stop=True)
                        nc.vector.tensor_copy(scol[:, 2 * p0:2 * p0 + 2], st_ps[:, 0:2])
                    # scale W rows by s
                    for p0 in range(kt):
                        wraw = dense.tile([P, 1408], F32, tag="wraw")
                        nc.sync.dma_start(wraw[:, :cols], w_r[i][p0 * P:(p0 + 1) * P, :])
                        nc.vector.tensor_scalar(
                            out=wt[:, p0 * 1408:p0 * 1408 + cols], in0=wraw[:, :cols],
                            scalar1=scol[:, 2 * p0:2 * p0 + 1], scalar2=None, op0=OP.mult)
                    # bias row = t @ W + const_row  (via matmuls into psum)
                    bps = []
                    for b in range(nb):
                        bpt = dpsum.tile([1, 448], F32, tag=f"hps{b}")
                        bps.append(bpt)
                    cro = dense.tile([1, 1408], F32, tag="cro")
                    nc.sync.dma_start(cro[:, :cols], cr_r[i][:, :])
                    for b in range(nb):
                        c0, c1 = b * 448, min((b + 1) * 448, cols)
                        for p0 in range(kt):
                            wraw2 = dense.tile([P, 448], F32, tag="wraw2")
                            nc.sync.dma_start(wraw2[:, :c1 - c0], w_r[i][p0 * P:(p0 + 1) * P, c0:c1])
                            nc.tensor.matmul(out=bps[b][:1, :c1 - c0],
                                             lhsT=scol[:, 2 * p0 + 1:2 * p0 + 2],
                                             rhs=wraw2[:, :c1 - c0],
                                             start=(p0 == 0), stop=False)
                        nc.tensor.matmul(out=bps[b][:1, :c1 - c0], lhsT=ones_row[:1, 0:1],
                                         rhs=cro[:, c0:c1], start=False, stop=True)
                        nc.vector.tensor_copy(bias_row[:, c0:c1], bps[b][:1, :c1 - c0])

                # ---- stage B: dense per node-chunk ----
                for ch in range(NCHUNK):
                    xc = dense.tile([P, din], F32, tag="xc")
                    nc.sync.dma_start(xc[:], x_in[ch * P:(ch + 1) * P, :din])
                    xT = dense.tile([P, 2 * P], F32, tag="xT")
                    for p0 in range(kt):
                        pr = min(P, din - p0 * P)
                        tps = dpsum.tile([P, P], F32, tag="tps")
                        nc.tensor.transpose(out=tps[:pr, :], in_=xc[:, p0 * P:p0 * P + pr],
                                            identity=identity[:])
                        nc.vector.tensor_copy(xT[:pr, p0 * P:(p0 + 1) * P], tps[:pr, :])
                    hps = []
                    for b in range(nb):
                        hpt = dpsum.tile([P, 448], F32, tag=f"hps{b}")
                        hps.append(hpt)
                    for b in range(nb):
                        c0, c1 = b * 448, min((b + 1) * 448, cols)
                        for p0 in range(kt):
                            pr = min(P, din - p0 * P)
                            nc.tensor.matmul(out=hps[b][:, :c1 - c0],
                                             lhsT=xT[:pr, p0 * P:(p0 + 1) * P],
                                             rhs=wt[:pr, p0 * 1408 + c0:p0 * 1408 + c1],
                                             start=(p0 == 0), stop=False)
                        nc.tensor.matmul(out=hps[b][:, :c1 - c0], lhsT=ones_row[:1, :],
                                         rhs=bias_row[:, c0:c1], start=False, stop=True)
                    hrow = dense.tile([P, 4 * cbk], F32, tag="hrow")
                    x0row = dense.tile([P, 256], F32, tag="x0row")
                    for b in range(nb):
                        c0, c1 = b * 448, min((b + 1) * 448, cols)
                        if c0 < 4 * cbk:
                            cc1 = min(c1, 4 * cbk)
                            nc.vector.tensor_copy(hrow[:, c0:cc1], hps[b][:, :cc1 - c0])
                        if c1 > 4 * cbk:
                            cc0 = max(c0, 4 * cbk)
                            nc.vector.tensor_copy(x0row[:, cc0 - 4 * cbk:c1 - 4 * cbk],
                                                  hps[b][:, cc0 - c0:c1 - c0])
                    # scatter H rows to table shard [(p,j) rows]
                    nc.sync.dma_start(
                        tshard[ch * P * 4:(ch + 1) * P * 4, :].rearrange(
                            "(p j) c -> p j c", j=4),
                        hrow[:].rearrange("p (j c) -> p j c", j=4))
                    nc.sync.dma_start(x0_buf[ch * P:(ch + 1) * P, :], x0row[:])
                # dummy rows: H=0, es/ed = -1e9
                dmy = dense.tile([4, rowlen], F32, tag="dmy")
                nc.vector.memset(dmy[:], 0.0)
                nc.vector.memset(dmy[:, hd:hd + 2 * heads], -1e9)
                nc.sync.dma_start(tshard[DUMMY:DUMMY + 4, :], dmy[:])

                # ---- stage C: AllGather table ----
                nc.gpsimd.collective_compute(
                    "AllGather", OP.bypass, replica_groups=rg,
                    ins=[tshard[:, :]], outs=[tfull[:, :]])

                # ---- stage D: edge phase ----
                stat_s = statp.tile([P, 256], F32, tag="ss")
                stat_q = statp.tile([P, 256], F32, tag="sq")
                if i < 2:
                    nc.vector.memset(stat_s[:], 0.0)
                    nc.vector.memset(stat_q[:], 0.0)
                tptr = 0
                segi = 0
                cur_ch = -1
                nseg = len(segs)
                for (chu, j, nt) in segs:
                    if chu != cur_ch:
                        cur_ch = chu
                        nrows = min(P, RPC - chu * P)
                        # ed for this chunk's nodes: [128, 4*heads]
                        edch = chnk.tile([P, 4 * heads], F32, tag="edch")
                        base = chu * P * 4
                        nc.sync.dma_start(
                            edch[:].rearrange("p (j h) -> p j h", j=4),
                            tshard[base:base + 4 * P, hd + heads:hd + 2 * heads].rearrange(
                                "(p j) h -> p j h", j=4))
                        x0a = chnk.tile([P, 256], F32, tag="x0a")
                        nc.sync.dma_start(x0a[:], x0_buf[chu * P:(chu + 1) * P, :])
                    aps = apsum.tile([P, hd + heads], F32, tag="aps")
                    for t in range(nt):
                        meta = edge.tile([P, 2], I32, tag="meta")
                        nc.sync.dma_start(meta[:], meta_r[tptr, :, :])
                        drow = edge.tile([1, P], F32, tag="drow")
                        nc.sync.dma_start(drow[:], dstrow_r[tptr:tptr + 1, :])
                        tptr += 1
                        rows = edge.tile([P, rowlen], F32, tag="rows")
                        nc.gpsimd.indirect_dma_start(
                            out=rows[:, :], out_offset=None, in_=tfull[:, :],
                            in_offset=bass.IndirectOffsetOnAxis(ap=meta[:, 0:1], axis=0))
                        hsl = rows[:, :hd]
                        essl = rows[:, hd:hd + heads]
                        # one-hot [e,d] and [d,e]
                        ohA = edge.tile([P, P], F32, tag="ohA")
                        nc.vector.tensor_scalar(out=ohA[:], in0=iota_row[:],
                                                scalar1=meta[:, 1:2].bitcast(F32),
                                                scalar2=None, op0=OP.is_equal)
                        bc = epsum.tile([P, P], F32, tag="bc")
                        nc.tensor.matmul(out=bc[:], lhsT=ones_row[:1, :], rhs=drow[:1, :],
                                         start=True, stop=True)
                        ohT = edge.tile([P, P], F32, tag="ohT")
                        nc.vector.tensor_scalar(out=ohT[:], in0=bc[:],
                                                scalar1=iota_col[:, :1],
                                                scalar2=None, op0=OP.is_equal)
                        edp = epsum.tile([P, heads], F32, tag="edp")
                        nc.tensor.matmul(out=edp[:], lhsT=ohT[:],
                                         rhs=edch[:, j * heads:(j + 1) * heads],
                                         start=True, stop=True)
                        lbuf = edge.tile([P, heads], F32, tag="lbuf")
                        nc.vector.tensor_tensor(out=lbuf[:], in0=essl, in1=edp[:], op=OP.add)
                        wb = edge.tile([P, heads], F32, tag="wb")
                        lneg = edge.tile([P, heads], F32, tag="lneg")
                        nc.vector.tensor_scalar(out=lneg[:], in0=lbuf[:], scalar1=0.0,
                                                scalar2=NEG, op0=OP.min, op1=OP.mult)
                        nc.vector.tensor_scalar(out=wb[:], in0=lbuf[:], scalar1=0.0,
                                                scalar2=None, op0=OP.max)
                        nc.vector.tensor_tensor(out=wb[:], in0=wb[:], in1=lneg[:], op=OP.add)
                        nc.scalar.activation(wb[:], wb[:], AF.Exp)
                        rhs = edge.tile([P, hd + heads], F32, tag="rhs")
                        dd = hd // heads
                        for h in range(heads):
                            nc.vector.tensor_scalar(
                                out=rhs[:, h * dd:(h + 1) * dd], in0=hsl[:, h * dd:(h + 1) * dd],
                                scalar1=wb[:, h:h + 1], scalar2=None, op0=OP.mult)
                        nc.vector.tensor_copy(rhs[:, hd:hd + heads], wb[:])
                        nc.tensor.matmul(out=aps[:], lhsT=ohA[:], rhs=rhs[:],
                                         start=(t == 0), stop=(t == nt - 1))
                    # post: x0a += msg/den
                    den = edge.tile([P, 2 * heads], F32, tag="den")
                    nc.vector.tensor_scalar(out=den[:, :heads], in0=aps[:, hd:hd + heads],
                                            scalar1=0.0, scalar2=None, op0=OP.is_equal)
                    nc.vector.tensor_tensor(out=den[:, :heads], in0=den[:, :heads],
                                            in1=aps[:, hd:hd + heads], op=OP.add)
                    nc.vector.reciprocal(den[:, heads:], den[:, :heads])
                    tmp = edge.tile([P, hd], F32, tag="tmp")
                    for h in range(heads):
                        nc.vector.tensor_scalar(
                            out=tmp[:, h * dd:(h + 1) * dd], in0=aps[:, h * dd:(h + 1) * dd],
                            scalar1=den[:, heads + h:heads + h + 1], scalar2=None, op0=OP.mult)
                    nc.vector.tensor_tensor(out=x0a[:, :hd], in0=x0a[:, :hd], in1=tmp[:],
                                            op=OP.add)
                    segi += 1
                    last_of_chunk = segi == nseg or segs[segi][0] != chu
                    if last_of_chunk:
                        nrows = min(P, RPC - chu * P)
                        nc.scalar.activation(x0a[:, :hd], x0a[:, :hd], AF.Relu)
                        if i < 2:
                            nc.sync.dma_start(x_out[chu * P:chu * P + nrows, :], x0a[:nrows, :])
                            nc.vector.tensor_tensor(out=stat_s[:nrows, :], in0=stat_s[:nrows, :],
                                                    in1=x0a[:nrows, :], op=OP.add)
                            sq = chnk.tile([P, 256], F32, tag="sq2")
                            nc.scalar.activation(sq[:nrows, :], x0a[:nrows, :], AF.Square)
                            nc.vector.tensor_tensor(out=stat_q[:nrows, :], in0=stat_q[:nrows, :],
                                                    in1=sq[:nrows, :], op=OP.add)
                        else:
                            nc.sync.dma_start(out_r[chu * P:chu * P + nrows, :], x0a[:nrows, :2])

                # ---- stats reduce + AllReduce (for next layer's BN) ----
                if i < 2:
                    sps = epsum.tile([1, 512], F32, tag="bc")
                    nc.tensor.matmul(out=sps[:1, 0:256], lhsT=ones_col[:, :1],
                                     rhs=stat_s[:], start=True, stop=True)
                    nc.tensor.matmul(out=sps[:1, 256:512], lhsT=ones_col[:, :1],
                                     rhs=stat_q[:], start=True, stop=True)
                    srow2 = statp.tile([1, 512], F32, tag="srow2")
                    nc.vector.tensor_copy(srow2[:], sps[:1, :])
                    nc.sync.dma_start(st_in[:, :], srow2[:])
                    nc.gpsimd.collective_compute(
                        "AllReduce", OP.add, replica_groups=rg,
                        ins=[st_in[:, :]], outs=[st_out[:, :]])

            for _ in range(reps):
                # zero x-buffer pad rows once per rep (pad nodes must stay finite/zero)
                zpad = dense.tile([RPAD - RPC, 256], F32, tag="zpad")
                nc.vector.memset(zpad[:], 0.0)
                for xb in x_bufs:
                    nc.sync.dma_start(xb[RPC:, :], zpad[:])
                for i in range(3):
                    layer(i)
    nc.compile()
    return nc


# --------------------------------------------------------------------------
# entry point
# --------------------------------------------------------------------------

def kernel(**inputs):
    x = np.asarray(inputs["x"], np.float32)
    edge_index = np.asarray(inputs["edge_index"])
    edge_class = np.asarray(inputs["edge_attr"])[:, -1]

    # layer-0 BN on host (x is an input; exact same math as reference)
    g0 = np.asarray(inputs["bng0"], np.float32)
    b0 = np.asarray(inputs["bnb0"], np.float32)
    mean, var = x.mean(0), x.var(0)
    xn0 = (x - mean) / np.sqrt(var + EPS) * g0 + b0
    xn0 = np.pad(xn0, ((0, 0), (0, 0))).astype(np.float32)

    segs, percore = _prep_edges(edge_index, edge_class)
    packs = _host_weights(inputs)

    key = tuple((a, b, c) for a, b, c in segs)
    if key not in _BUILD_CACHE:
        _BUILD_CACHE[key] = build_kernel(segs)
    nc = _BUILD_CACHE[key]

    T = sum(t for _, _, t in segs)
    in_maps = []
    for c in range(NCORES):
        xs = np.zeros((RPAD, F_IN), np.float32)
        xs[:RPC] = xn0[c * RPC:(c + 1) * RPC]
        meta = np.zeros((T, P, 2), np.int32)
        meta[:, :, 0] = percore[c]["rowidx"]
        meta[:, :, 1] = percore[c]["dstrow"].view(np.int32)
        m = {
            "xn0": xs,
            "meta": meta,
            "dstrow": percore[c]["dstrow"],
        }
        for i in range(3):
            m[f"wext{i}"] = packs[i][0]
            m[f"crow{i}"] = packs[i][1]
        for i in (1, 2):
            m[f"bn{i}"] = np.concatenate([
                np.asarray(inputs[f"bng{i}"], np.float32),
                np.asarray(inputs[f"bnb{i}"], np.float32)])[None, :]
        in_maps.append(m)

    res = run_bass_kernel_spmd(nc, in_maps=in_maps, core_ids=list(range(NCORES)))
    out = np.zeros((N, 2), np.float32)
    for c in range(NCORES):
        out[c * RPC:(c + 1) * RPC] = res.results[c]["out"][:RPC]
    return out

